# revision 15
# baseline (speedup 1.0000x reference)
"""DeepFM forward on 8 Trainium2 NeuronCores (Bass/Tile, SPMD).

Strategy: batch-shard all heavy work across the 8 cores (512 rows each).
The embedding gather (B*F = 106496 rows of 68B from a 354MB table) runs as
per-core indirect DMAs; FM + first MLP layer are fused into the same launch.
BatchNorm (training mode) needs full-batch statistics, which couples the
batch shards between layers; device collectives on this platform cost
~80us each, so the pipeline is split into 3 small SPMD launches and the
8-way partial-sum reduction of the BN statistics (512 + 256 floats) is done
on the host between launches. The L1/L2 biases cancel inside BatchNorm and
are dropped entirely; b3 and fm_bias are folded into the FM output on host.
"""
import os
import numpy as np

import concourse.bass as bass
import concourse.bacc as bacc
import concourse.tile as tile
import concourse.mybir as mybir
from concourse.bass_utils import run_bass_kernel_spmd
from concourse.library_config import mlp as mlp_lib

B, F, V, D = 4096, 26, 200000, 16
E = D + 1              # 17 floats per combined table row (16 emb + 1 lin)
H1, H2 = 256, 128
EPS = 1e-5
NCORES = 8
BS = B // NCORES       # 512 rows per core
NBB = BS // 128        # 4 batch sub-blocks of 128 (partition dim)
NJ = NBB * F           # 104 gather slots per partition
P = 128
FP = mybir.dt.float32
GS = 7                 # table entries packed per 512B gather row
RPF = (V + GS - 1) // GS   # 28572 rows per field (fits int16)
REW = 128              # f32 per packed row: 7*16 emb + 7 lin + 9 pad
GATHER = os.environ.get("BASS_DEEPFM_GATHER", "dmagather")

_cache = {}
LAST_EXEC_NS = []      # per-launch exec_time_ns when profiling is enabled


def _profiling():
    return os.environ.get("BASS_DEEPFM_PROFILE", "") == "1"


def _install_profile_shim():
    """Register the NTFF profile hook so run_bass_kernel_spmd(trace=True)
    returns exec_time_ns under axon. Best-effort."""
    import sys
    import types
    try:
        import antenv.axon_hooks  # noqa: F401
    except ImportError:
        mod = types.ModuleType("antenv.axon_hooks")
        _h = [None]
        mod.set_axon_ntff_profile_hook = lambda h: _h.__setitem__(0, h)
        mod.get_axon_ntff_profile_hook = lambda: _h[0]
        sys.modules["antenv.axon_hooks"] = mod
        import antenv
        antenv.axon_hooks = mod
    try:
        from antenv.axon_hooks import (
            get_axon_ntff_profile_hook,
            set_axon_ntff_profile_hook,
        )
        if get_axon_ntff_profile_hook() is None:
            from trn_agent_boot.trn_boot import _ntff_profile_via_ctypes
            set_axon_ntff_profile_hook(
                _ntff_profile_via_ctypes("/opt/axon/libaxon_pjrt.so"))
        import concourse.bass_utils as bu
        bu.upload_artifacts = lambda tmpdir: "local://skipped"
        return True
    except Exception:
        return False


def _run(nc, in_maps):
    trace = _profiling() and _install_profile_shim()
    res = run_bass_kernel_spmd(nc, in_maps, list(range(NCORES)), trace=trace)
    if trace:
        LAST_EXEC_NS.append(res.exec_time_ns)
    return res.results


# ---------------------------------------------------------------- launch 1
# gather + FM part + transpose + MLP layer 1 + BN1 partial stats
def _build_launch1():
    nc = bacc.Bacc("TRN2", target_bir_lowering=False, debug=False,
                   num_devices=NCORES)
    tbl = nc.dram_tensor("tbl", [F * V, E], FP, kind="ExternalInput")
    idx = nc.dram_tensor("idx", [P, NJ], mybir.dt.int32, kind="ExternalInput")
    w1t = nc.dram_tensor("w1t", [F * D, H1], FP, kind="ExternalInput")
    fm_o = nc.dram_tensor("fm", [P, NBB], FP, kind="ExternalOutput")
    st_o = nc.dram_tensor("st1", [P, 4], FP, kind="ExternalOutput")
    h1_o = nc.dram_tensor("h1", [P, 2 * BS], FP, kind="ExternalOutput")

    from concourse.masks import make_identity

    with tile.TileContext(nc) as tc:
        with (
            tc.tile_pool(name="sb", bufs=1) as sb,
            tc.tile_pool(name="pt", bufs=4, space="PSUM") as pt,
            tc.tile_pool(name="ph", bufs=2, space="PSUM") as ph,
        ):
            idx_t = sb.tile([P, NJ], mybir.dt.int32)
            nc.sync.dma_start(idx_t[:], idx[:, :])
            ident = sb.tile([P, P], FP)
            make_identity(nc, ident[:])
            w1k = []
            for k in range(4):
                kk = min(128, F * D - 128 * k)
                t = sb.tile([P, H1], FP, tag=f"w1k{k}", name=f"w1k{k}")
                nc.sync.dma_start(t[0:kk, :], w1t[128 * k:128 * k + kk, :])
                w1k.append((t, kk))

            G = sb.tile([P, NJ, E], FP)
            for j in range(NJ):
                nc.gpsimd.indirect_dma_start(
                    out=G[:, j, :],
                    out_offset=None,
                    in_=tbl[:, :],
                    in_offset=bass.IndirectOffsetOnAxis(
                        ap=idx_t[:, j:j + 1], axis=0),
                )

            # compact, contiguous emb-only copy (drops the lin column)
            Gemb = sb.tile([P, NJ, D], FP)
            nc.vector.tensor_copy(Gemb[:], G[:, :, 0:D])

            # ---- FM part (on the gathered, batch-major layout) ----
            sq = sb.tile([P, F, D], FP)          # scratch for squares
            ssq = sb.tile([P, NBB], FP)          # sum_{f,d} e^2 per row
            for bb in range(NBB):
                nc.scalar.activation(
                    out=sq[:],
                    in_=Gemb[:, bb * F:(bb + 1) * F, :],
                    func=mybir.ActivationFunctionType.Square,
                    accum_out=ssq[:, bb:bb + 1],
                )
            s = sb.tile([P, NBB, D], FP)         # sum_f e
            Gd = Gemb[:, :, :].rearrange("p (bb f) d -> p bb d f", f=F)
            nc.vector.reduce_sum(s[:], Gd, axis=mybir.AxisListType.X)
            lin = sb.tile([P, NBB], FP)          # sum_f lin
            Gl = G[:, :, D:E].rearrange("p (bb f) e -> p bb (f e)", f=F)
            nc.vector.reduce_sum(lin[:], Gl, axis=mybir.AxisListType.X)
            s2 = sb.tile([P, NBB, D], FP)
            nc.vector.tensor_tensor(out=s2[:], in0=s[:], in1=s[:],
                                    op=mybir.AluOpType.mult)
            s2r = sb.tile([P, NBB], FP)
            nc.vector.reduce_sum(s2r[:], s2[:], axis=mybir.AxisListType.X)
            t1 = sb.tile([P, NBB], FP)
            nc.vector.tensor_tensor(out=t1[:], in0=s2r[:], in1=ssq[:],
                                    op=mybir.AluOpType.subtract)
            fmh = sb.tile([P, NBB], FP)
            nc.vector.tensor_scalar(out=fmh[:], in0=t1[:], scalar1=0.5,
                                    scalar2=None, op0=mybir.AluOpType.mult)
            fmv = sb.tile([P, NBB], FP)
            nc.vector.tensor_tensor(out=fmv[:], in0=fmh[:], in1=lin[:],
                                    op=mybir.AluOpType.add)
            nc.sync.dma_start(fm_o[:, :], fmv[:])

            # ---- transpose h (batch-major) -> hT (feature-major) ----
            hT = []
            for r in range(4):
                hT.append(sb.tile([P, BS], FP, tag=f"hT{r}", name=f"hT{r}"))
            for bb in range(NBB):
                for r in range(4):
                    nf = 8 if r < 3 else 2       # fields per 128-row chunk
                    nrow = nf * D
                    blk = Gemb[:, bb * F + 8 * r: bb * F + 8 * r + nf, :]
                    blk = blk.rearrange("p f d -> p (f d)")
                    ptt = pt.tile([P, P], FP, tag="pt")
                    nc.tensor.transpose(out=ptt[0:nrow, :], in_=blk,
                                        identity=ident[:])
                    nc.scalar.copy(
                        hT[r][0:nrow, bb * P:(bb + 1) * P], ptt[0:nrow, :])

            # ---- layer 1 matmul + BN1 partial stats ----
            stt = sb.tile([P, 4], FP)
            sq1 = sb.tile([P, BS], FP)
            h1sb = sb.tile([P, 2, BS], FP)
            for m in range(2):
                pm = ph.tile([P, BS], FP, tag="ph")
                for i, k in enumerate(GORDER):
                    _, kk = w1k[k]
                    nc.tensor.matmul(
                        out=pm[:],
                        lhsT=w1k[k][0][0:kk, m * 128:(m + 1) * 128],
                        rhs=hT[k][0:kk, :],
                        start=(i == 0), stop=(i == 3),
                    )
                nc.vector.reduce_sum(stt[:, m:m + 1], pm[:],
                                     axis=mybir.AxisListType.X)
                nc.scalar.activation(
                    out=sq1[:], in_=pm[:],
                    func=mybir.ActivationFunctionType.Square,
                    accum_out=stt[:, 2 + m:3 + m],
                )
                nc.vector.tensor_copy(h1sb[:, m, :], pm[:])
            nc.sync.dma_start(st_o[:, :], stt[:])
            nc.sync.dma_start(h1_o[:, :],
                              h1sb[:].rearrange("p a b -> p (a b)"))
    nc.compile()
    return nc



# ------------------------------------------------- launch 1 (dma_gather)
# Same outputs as _build_launch1, but the gather runs as 26 per-field
# dma_gather calls (512 idxs each) spread over 4 SWDGE queues. Each 512B
# table row packs 7 vocab entries (emb + lin); the entry-within-row (v%7)
# is selected on-device with 7 predicated copies driven by host-built masks.
def _build_launch1_dg():
    nc = bacc.Bacc("TRN2", target_bir_lowering=False, debug=False,
                   num_devices=NCORES, num_swdge_queues=4)
    tbl = nc.dram_tensor("tbl", [F * RPF, REW], FP, kind="ExternalInput")
    idx = nc.dram_tensor("idx", [P, F * 32], mybir.dt.int16,
                         kind="ExternalInput")
    mke = nc.dram_tensor("mke", [P, GS, NJ, D], mybir.dt.uint8, kind="ExternalInput")
    mkl = nc.dram_tensor("mkl", [P, GS, NJ], FP, kind="ExternalInput")
    w1t = nc.dram_tensor("w1t", [F * D, H1], FP, kind="ExternalInput")
    idn = nc.dram_tensor("idn", [P, P], FP, kind="ExternalInput")
    fm_o = nc.dram_tensor("fm", [P, NBB], FP, kind="ExternalOutput")
    st_o = nc.dram_tensor("st1", [P, 4], FP, kind="ExternalOutput")
    h1_o = nc.dram_tensor("h1", [P, 2 * BS], FP, kind="ExternalOutput")

    with tile.TileContext(nc) as tc:
        with (
            tc.tile_pool(name="sb", bufs=1) as sb,
            tc.tile_pool(name="pt", bufs=4, space="PSUM") as pt,
            tc.tile_pool(name="ph", bufs=2, space="PSUM") as ph,
        ):
            lib_inst = nc.gpsimd.load_library(mlp_lib)
            idx_t = sb.tile([P, F * 32], mybir.dt.int16)
            nc.sync.dma_start(idx_t[:], idx[:, :])
            mke_t = sb.tile([P, GS, NJ, D], mybir.dt.uint8)
            nc.sync.dma_start(mke_t[:].rearrange("p a b c -> p (a b c)"),
                              mke[:, :, :, :].rearrange("p a b c -> p (a b c)"))
            mkl_t = sb.tile([P, GS, NJ], FP)
            nc.sync.dma_start(mkl_t[:].rearrange("p a b -> p (a b)"),
                              mkl[:, :, :].rearrange("p a b -> p (a b)"))
            ident = sb.tile([P, P], FP)
            nc.sync.dma_start(ident[:], idn[:, :])
            w1k = []
            for k in range(4):
                kk = min(128, F * D - 128 * k)
                t = sb.tile([P, H1], FP, tag=f"w1k{k}", name=f"w1k{k}")
                nc.sync.dma_start(t[0:kk, :], w1t[128 * k:128 * k + kk, :])
                w1k.append((t, kk))

            # gathers grouped by transpose chunk r (fields 8r..8r+8) so the
            # per-group select/transpose pipeline can start before all
            # fields have landed. Queue assignment rotates within a group.
            GRPS = [(0, 8), (8, 8), (16, 8), (24, 2)]
            GORDER = [3, 0, 1, 2]
            G7g = []
            for r, (f0, nf) in enumerate(GRPS):
                G7g.append(sb.tile([P, nf, NBB, REW], FP, tag=f"G7g{r}",
                                   name=f"G7g{r}"))
            qn = 0
            for r in GORDER:
                f0, nf = GRPS[r]
                for fl in range(nf):
                    f = f0 + fl
                    gi = nc.gpsimd.dma_gather(
                        G7g[r][:, fl, :, :],
                        tbl[f * RPF:(f + 1) * RPF, :],
                        idx_t[:, f * 32:(f + 1) * 32],
                        BS, BS, REW,
                        single_packet=False,
                        queue_num=qn % 4,
                    )
                    qn += 1
                    tile.add_dep_helper(gi.ins, lib_inst.ins,
                                        reason="dma_gather after lib load")

            # ---- slot select, per field-group: E_r[p, bb, f, d] ----
            Eg = []
            for r, (f0, nf) in enumerate(GRPS):
                Eg.append(sb.tile([P, NBB, nf, D], FP, tag=f"Eg{r}",
                                  name=f"Eg{r}"))
            mkev = mke_t[:, :, :, :].rearrange(
                "p s (bb f) d -> p s f bb d", f=F)
            linp = []
            for r in GORDER:
                f0, nf = GRPS[r]
                Erv = Eg[r][:, :, :, :].rearrange("p bb f d -> p f bb d")
                for sslot in range(GS):
                    nc.vector.copy_predicated(
                        out=Erv,
                        mask=mkev[:, sslot, f0:f0 + nf, :, :],
                        data=G7g[r][:, :, :, sslot * D:(sslot + 1) * D],
                    )
            # lin: mask-weighted sum over the 7 slots (contiguous inner dim)
            mklg = mkl_t[:, :, :].rearrange("p s (bb f) -> p f bb s", f=F)
            for r in GORDER:
                f0, nf = GRPS[r]
                lm = sb.tile([P, nf, NBB, GS], FP, tag=f"lm{r}",
                             name=f"lm{r}")
                nc.vector.tensor_tensor(
                    out=lm[:],
                    in0=G7g[r][:, :, :, GS * D:GS * D + GS],
                    in1=mklg[:, f0:f0 + nf, :, :],
                    op=mybir.AluOpType.mult)
                ls = sb.tile([P, nf, NBB], FP, tag=f"ls{r}", name=f"ls{r}")
                nc.vector.reduce_sum(ls[:], lm[:], axis=mybir.AxisListType.X)
                lr = sb.tile([P, NBB], FP, tag=f"lr{r}", name=f"lr{r}")
                nc.vector.reduce_sum(
                    lr[:], ls[:, :, :].rearrange("p f bb -> p bb f"),
                    axis=mybir.AxisListType.X)
                linp.append(lr)

            # ---- FM part (partials per group so they pipeline) ----
            sq = sb.tile([P, F, D], FP)
            ssqp = sb.tile([P, NBB, 4], FP)
            sp = sb.tile([P, 4, NBB, D], FP)
            for r, (f0, nf) in enumerate(GRPS):
                for bb in range(NBB):
                    nc.scalar.activation(
                        out=sq[:, 0:nf, :],
                        in_=Eg[r][:, bb, :, :],
                        func=mybir.ActivationFunctionType.Square,
                        accum_out=ssqp[:, bb, r:r + 1],
                    )
                nc.vector.reduce_sum(
                    sp[:, r, :, :],
                    Eg[r][:, :, :, :].rearrange("p bb f d -> p bb d f"),
                    axis=mybir.AxisListType.X)
            ssq = sb.tile([P, NBB], FP)
            nc.vector.reduce_sum(ssq[:], ssqp[:], axis=mybir.AxisListType.X)
            s01 = sb.tile([P, NBB, D], FP)
            nc.vector.tensor_tensor(out=s01[:], in0=sp[:, 0, :, :],
                                    in1=sp[:, 1, :, :],
                                    op=mybir.AluOpType.add)
            s23 = sb.tile([P, NBB, D], FP)
            nc.vector.tensor_tensor(out=s23[:], in0=sp[:, 2, :, :],
                                    in1=sp[:, 3, :, :],
                                    op=mybir.AluOpType.add)
            s = sb.tile([P, NBB, D], FP)
            nc.vector.tensor_tensor(out=s[:], in0=s01[:], in1=s23[:],
                                    op=mybir.AluOpType.add)
            l01 = sb.tile([P, NBB], FP)
            nc.vector.tensor_tensor(out=l01[:], in0=linp[0][:], in1=linp[1][:],
                                    op=mybir.AluOpType.add)
            l23 = sb.tile([P, NBB], FP)
            nc.vector.tensor_tensor(out=l23[:], in0=linp[2][:], in1=linp[3][:],
                                    op=mybir.AluOpType.add)
            lin = sb.tile([P, NBB], FP)
            nc.vector.tensor_tensor(out=lin[:], in0=l01[:], in1=l23[:],
                                    op=mybir.AluOpType.add)
            s2 = sb.tile([P, NBB, D], FP)
            nc.vector.tensor_tensor(out=s2[:], in0=s[:], in1=s[:],
                                    op=mybir.AluOpType.mult)
            s2r = sb.tile([P, NBB], FP)
            nc.vector.reduce_sum(s2r[:], s2[:], axis=mybir.AxisListType.X)
            t1 = sb.tile([P, NBB], FP)
            nc.vector.tensor_tensor(out=t1[:], in0=s2r[:], in1=ssq[:],
                                    op=mybir.AluOpType.subtract)
            fmh = sb.tile([P, NBB], FP)
            nc.vector.tensor_scalar(out=fmh[:], in0=t1[:], scalar1=0.5,
                                    scalar2=None, op0=mybir.AluOpType.mult)
            fmv = sb.tile([P, NBB], FP)
            nc.vector.tensor_tensor(out=fmv[:], in0=fmh[:], in1=lin[:],
                                    op=mybir.AluOpType.add)
            nc.sync.dma_start(fm_o[:, :], fmv[:])

            # ---- transpose to feature-major ----
            hT = []
            for r in range(4):
                hT.append(sb.tile([P, BS], FP, tag=f"hT{r}", name=f"hT{r}"))
            for r, (f0, nf) in enumerate(GRPS):
                nrow = nf * D
                for bb in range(NBB):
                    blk = Eg[r][:, bb, :, :].rearrange("p f d -> p (f d)")
                    ptt = pt.tile([P, P], FP, tag="pt")
                    nc.tensor.transpose(out=ptt[0:nrow, :], in_=blk,
                                        identity=ident[:])
                    nc.scalar.copy(
                        hT[r][0:nrow, bb * P:(bb + 1) * P], ptt[0:nrow, :])

            # ---- layer 1 + BN1 partial stats ----
            stt = sb.tile([P, 4], FP)
            sq1 = sb.tile([P, BS], FP)
            h1sb = sb.tile([P, 2, BS], FP)
            for m in range(2):
                pm = ph.tile([P, BS], FP, tag="ph")
                for i, k in enumerate(GORDER):
                    _, kk = w1k[k]
                    nc.tensor.matmul(
                        out=pm[:],
                        lhsT=w1k[k][0][0:kk, m * 128:(m + 1) * 128],
                        rhs=hT[k][0:kk, :],
                        start=(i == 0), stop=(i == 3),
                    )
                nc.vector.reduce_sum(stt[:, m:m + 1], pm[:],
                                     axis=mybir.AxisListType.X)
                nc.scalar.activation(
                    out=sq1[:], in_=pm[:],
                    func=mybir.ActivationFunctionType.Square,
                    accum_out=stt[:, 2 + m:3 + m],
                )
                nc.vector.tensor_copy(h1sb[:, m, :], pm[:])
            nc.sync.dma_start(st_o[:, :], stt[:])
            nc.sync.dma_start(h1_o[:, :],
                              h1sb[:].rearrange("p a b -> p (a b)"))
    nc.compile()
    return nc


# ---------------------------------------------------------------- launch 2
# BN1 (global stats) + relu + MLP layer 2 + BN2 partial stats
def _build_launch2():
    nc = bacc.Bacc("TRN2", target_bir_lowering=False, debug=False,
                   num_devices=NCORES)
    h1 = nc.dram_tensor("h1", [P, 2 * BS], FP, kind="ExternalInput")
    st1 = nc.dram_tensor("st1", [P, 4], FP, kind="ExternalInput")
    g1 = nc.dram_tensor("g1", [P, 2], FP, kind="ExternalInput")
    be1 = nc.dram_tensor("be1", [P, 2], FP, kind="ExternalInput")
    w2t = nc.dram_tensor("w2t", [H1, H2], FP, kind="ExternalInput")
    h2_o = nc.dram_tensor("h2", [P, BS], FP, kind="ExternalOutput")
    st_o = nc.dram_tensor("st2", [P, 2], FP, kind="ExternalOutput")

    with tile.TileContext(nc) as tc:
        with (
            tc.tile_pool(name="sb", bufs=1) as sb,
            tc.tile_pool(name="ph", bufs=1, space="PSUM") as ph,
        ):
            h1s = sb.tile([P, 2, BS], FP)
            nc.sync.dma_start(h1s[:].rearrange("p a b -> p (a b)"), h1[:, :])
            sts = sb.tile([P, 4], FP)
            nc.sync.dma_start(sts[:], st1[:, :])
            g1s = sb.tile([P, 2], FP)
            nc.sync.dma_start(g1s[:], g1[:, :])
            be1s = sb.tile([P, 2], FP)
            nc.sync.dma_start(be1s[:], be1[:, :])
            w2k = []
            for k in range(2):
                t = sb.tile([P, H2], FP, tag=f"w2k{k}", name=f"w2k{k}")
                nc.sync.dma_start(t[:], w2t[128 * k:128 * (k + 1), :])
                w2k.append(t)

            # a = g/sqrt(var+eps), b' = be - a*mean  (both [128, 2])
            mean = sb.tile([P, 2], FP)
            nc.vector.tensor_scalar_mul(mean[:], sts[:, 0:2], 1.0 / B)
            q = sb.tile([P, 2], FP)
            nc.vector.tensor_scalar_mul(q[:], sts[:, 2:4], 1.0 / B)
            m2 = sb.tile([P, 2], FP)
            nc.vector.tensor_tensor(out=m2[:], in0=mean[:], in1=mean[:],
                                    op=mybir.AluOpType.mult)
            var = sb.tile([P, 2], FP)
            nc.vector.tensor_tensor(out=var[:], in0=q[:], in1=m2[:],
                                    op=mybir.AluOpType.subtract)
            vare = sb.tile([P, 2], FP)
            nc.vector.tensor_scalar_add(vare[:], var[:], EPS)
            sd = sb.tile([P, 2], FP)
            nc.scalar.activation(sd[:], vare[:],
                                 mybir.ActivationFunctionType.Sqrt)
            rsd = sb.tile([P, 2], FP)
            nc.vector.reciprocal(rsd[:], sd[:])
            a1 = sb.tile([P, 2], FP)
            nc.vector.tensor_tensor(out=a1[:], in0=g1s[:], in1=rsd[:],
                                    op=mybir.AluOpType.mult)
            am = sb.tile([P, 2], FP)
            nc.vector.tensor_tensor(out=am[:], in0=a1[:], in1=mean[:],
                                    op=mybir.AluOpType.mult)
            b1p = sb.tile([P, 2], FP)
            nc.vector.tensor_tensor(out=b1p[:], in0=be1s[:], in1=am[:],
                                    op=mybir.AluOpType.subtract)

            h1n = sb.tile([P, 2, BS], FP)
            for m in range(2):
                nc.scalar.activation(
                    out=h1n[:, m, :], in_=h1s[:, m, :],
                    func=mybir.ActivationFunctionType.Relu,
                    bias=b1p[:, m:m + 1], scale=a1[:, m:m + 1],
                )

            pm = ph.tile([P, BS], FP)
            for k in range(2):
                nc.tensor.matmul(out=pm[:], lhsT=w2k[k][:],
                                 rhs=h1n[:, k, :],
                                 start=(k == 0), stop=(k == 1))
            stt = sb.tile([P, 2], FP)
            nc.vector.reduce_sum(stt[:, 0:1], pm[:],
                                 axis=mybir.AxisListType.X)
            sq2 = sb.tile([P, BS], FP)
            nc.scalar.activation(out=sq2[:], in_=pm[:],
                                 func=mybir.ActivationFunctionType.Square,
                                 accum_out=stt[:, 1:2])
            h2sb = sb.tile([P, BS], FP)
            nc.vector.tensor_copy(h2sb[:], pm[:])
            nc.sync.dma_start(st_o[:, :], stt[:])
            nc.sync.dma_start(h2_o[:, :], h2sb[:])
    nc.compile()
    return nc


# ---------------------------------------------------------------- launch 3
# BN2 (global stats) + relu + layer 3 + merge FM + sigmoid
def _build_launch3():
    nc = bacc.Bacc("TRN2", target_bir_lowering=False, debug=False,
                   num_devices=NCORES)
    h2 = nc.dram_tensor("h2", [P, BS], FP, kind="ExternalInput")
    st2 = nc.dram_tensor("st2", [P, 2], FP, kind="ExternalInput")
    g2 = nc.dram_tensor("g2", [P, 1], FP, kind="ExternalInput")
    be2 = nc.dram_tensor("be2", [P, 1], FP, kind="ExternalInput")
    w3t = nc.dram_tensor("w3t", [H2, 1], FP, kind="ExternalInput")
    fmx = nc.dram_tensor("fmx", [P, NBB], FP, kind="ExternalInput")
    out_o = nc.dram_tensor("out", [P, NBB], FP, kind="ExternalOutput")

    with tile.TileContext(nc) as tc:
        with (
            tc.tile_pool(name="sb", bufs=1) as sb,
            tc.tile_pool(name="ph", bufs=1, space="PSUM") as ph,
        ):
            h2s = sb.tile([P, BS], FP)
            nc.sync.dma_start(h2s[:], h2[:, :])
            sts = sb.tile([P, 2], FP)
            nc.sync.dma_start(sts[:], st2[:, :])
            g2s = sb.tile([P, 1], FP)
            nc.sync.dma_start(g2s[:], g2[:, :])
            be2s = sb.tile([P, 1], FP)
            nc.sync.dma_start(be2s[:], be2[:, :])
            w3s = sb.tile([P, 1], FP)
            nc.sync.dma_start(w3s[:], w3t[:, :])
            fms = sb.tile([P, NBB], FP)
            nc.sync.dma_start(fms[:], fmx[:, :])

            mean = sb.tile([P, 1], FP)
            nc.vector.tensor_scalar_mul(mean[:], sts[:, 0:1], 1.0 / B)
            q = sb.tile([P, 1], FP)
            nc.vector.tensor_scalar_mul(q[:], sts[:, 1:2], 1.0 / B)
            m2 = sb.tile([P, 1], FP)
            nc.vector.tensor_tensor(out=m2[:], in0=mean[:], in1=mean[:],
                                    op=mybir.AluOpType.mult)
            var = sb.tile([P, 1], FP)
            nc.vector.tensor_tensor(out=var[:], in0=q[:], in1=m2[:],
                                    op=mybir.AluOpType.subtract)
            vare = sb.tile([P, 1], FP)
            nc.vector.tensor_scalar_add(vare[:], var[:], EPS)
            sd = sb.tile([P, 1], FP)
            nc.scalar.activation(sd[:], vare[:],
                                 mybir.ActivationFunctionType.Sqrt)
            rsd = sb.tile([P, 1], FP)
            nc.vector.reciprocal(rsd[:], sd[:])
            a2 = sb.tile([P, 1], FP)
            nc.vector.tensor_tensor(out=a2[:], in0=g2s[:], in1=rsd[:],
                                    op=mybir.AluOpType.mult)
            am = sb.tile([P, 1], FP)
            nc.vector.tensor_tensor(out=am[:], in0=a2[:], in1=mean[:],
                                    op=mybir.AluOpType.mult)
            b2p = sb.tile([P, 1], FP)
            nc.vector.tensor_tensor(out=b2p[:], in0=be2s[:], in1=am[:],
                                    op=mybir.AluOpType.subtract)

            h2n = sb.tile([P, BS], FP)
            nc.scalar.activation(out=h2n[:], in_=h2s[:],
                                 func=mybir.ActivationFunctionType.Relu,
                                 bias=b2p[:, 0:1], scale=a2[:, 0:1])

            p3 = ph.tile([P, NBB], FP)
            for bb in range(NBB):
                nc.tensor.matmul(out=p3[:, bb:bb + 1],
                                 lhsT=h2n[:, bb * P:(bb + 1) * P],
                                 rhs=w3s[:], start=True, stop=True)
            logit = sb.tile([P, NBB], FP)
            nc.vector.tensor_tensor(out=logit[:], in0=p3[:], in1=fms[:],
                                    op=mybir.AluOpType.add)
            outs = sb.tile([P, NBB], FP)
            nc.scalar.activation(outs[:], logit[:],
                                 mybir.ActivationFunctionType.Sigmoid)
            nc.sync.dma_start(out_o[:, :], outs[:])
    nc.compile()
    return nc


def kernel(**inputs):
    LAST_EXEC_NS.clear()
    x = np.asarray(inputs["x"]).astype(np.int64)            # [B, F]
    emb = np.asarray(inputs["emb_tables"], dtype=np.float32)  # [F, V, D]
    lint = np.asarray(inputs["lin_tables"], dtype=np.float32)  # [F, V, 1]
    fm_bias = float(np.asarray(inputs["fm_bias"]).reshape(-1)[0])
    W1 = np.asarray(inputs["W1"], dtype=np.float32)
    g1 = np.asarray(inputs["g1"], dtype=np.float32)
    be1 = np.asarray(inputs["be1"], dtype=np.float32)
    W2 = np.asarray(inputs["W2"], dtype=np.float32)
    g2 = np.asarray(inputs["g2"], dtype=np.float32)
    be2 = np.asarray(inputs["be2"], dtype=np.float32)
    W3 = np.asarray(inputs["W3"], dtype=np.float32)
    b3 = float(np.asarray(inputs["b3"]).reshape(-1)[0])

    xs = x.reshape(NCORES, NBB, P, F)                       # (c, bb, p, f)
    if GATHER == "dmagather":
        # packed table: row r of field f holds entries v in [7r, 7r+7):
        # 7*16 emb floats, then 7 lin floats, padded to 128 floats (512B)
        Vp = RPF * GS
        pe = np.zeros((F, Vp, D), np.float32)
        pe[:, :V] = emb
        pl = np.zeros((F, Vp), np.float32)
        pl[:, :V] = lint.reshape(F, V)
        TBL = np.zeros((F * RPF, REW), np.float32)
        TBL[:, :GS * D] = pe.reshape(F * RPF, GS * D)
        TBL[:, GS * D:GS * D + GS] = pl.reshape(F * RPF, GS)
        del pe, pl
        rows = (x // GS).astype(np.int16)                   # [B, F]
        slots = (x % GS).astype(np.int8)                    # [B, F]
        rs = rows.reshape(NCORES, NBB, P, F)
        ss = slots.reshape(NCORES, NBB, P, F)
        idx_all, mke_all, mkl_all = [], [], []
        for c in range(NCORES):
            cols = []
            for f in range(F):
                a = rs[c, :, :, f].reshape(BS)              # i = bb*128+p
                w = a.reshape(32, 16).T                     # [16, 32]
                cols.append(np.tile(w, (8, 1)))             # [128, 32]
            idx_all.append(np.ascontiguousarray(
                np.concatenate(cols, axis=1)))              # [128, F*32]
            sl = ss[c].transpose(1, 0, 2).reshape(P, NJ)    # [p, (bb f)]
            m = (sl[:, None, :] ==
                 np.arange(GS, dtype=np.int8)[None, :, None])  # [p, 7, NJ]
            mkl_all.append(np.ascontiguousarray(m.astype(np.float32)))
            mke_all.append(np.ascontiguousarray(
                np.broadcast_to(m[:, :, :, None],
                                (P, GS, NJ, D)).astype(np.uint8)))
    else:
        # combined table [F*V, 17]; row (f, v) = emb[f, v, :] ++ lin[f, v]
        C = np.concatenate([emb, lint], axis=2).reshape(F * V, E)
        foff = (np.arange(F, dtype=np.int64) * V)[None, None, :]
        idx_all = []
        for c in range(NCORES):
            a = xs[c] + foff                                # [bb, p, f]
            a = a.transpose(1, 0, 2).reshape(P, NJ)         # [p, bb*F]
            idx_all.append(np.ascontiguousarray(a.astype(np.int32)))

    W1T = np.ascontiguousarray(W1.T)                        # [416, 256]
    W2T = np.ascontiguousarray(W2.T)                        # [256, 128]
    W3T = np.ascontiguousarray(W3.reshape(1, H2).T)         # [128, 1]
    g1d = np.ascontiguousarray(g1.reshape(2, P).T)          # [128, 2]
    be1d = np.ascontiguousarray(be1.reshape(2, P).T)
    g2d = np.ascontiguousarray(g2.reshape(1, P).T)          # [128, 1]
    be2d = np.ascontiguousarray(be2.reshape(1, P).T)

    if GATHER == "dmagather":
        if "l1" not in _cache:
            _cache["l1"] = _build_launch1_dg()
        idn = np.eye(P, dtype=np.float32)
        r1 = _run(_cache["l1"],
                  [{"tbl": TBL, "idx": idx_all[c], "mke": mke_all[c],
                    "mkl": mkl_all[c], "w1t": W1T, "idn": idn}
                   for c in range(NCORES)])
    else:
        if "l1" not in _cache:
            _cache["l1"] = _build_launch1()
        r1 = _run(_cache["l1"],
                  [{"tbl": C, "idx": idx_all[c], "w1t": W1T}
                   for c in range(NCORES)])

    st1g = np.sum([r1[c]["st1"] for c in range(NCORES)], axis=0)
    if "l2" not in _cache:
        _cache["l2"] = _build_launch2()
    r2 = _run(_cache["l2"],
              [{"h1": r1[c]["h1"], "st1": st1g, "g1": g1d, "be1": be1d,
                "w2t": W2T} for c in range(NCORES)])

    st2g = np.sum([r2[c]["st2"] for c in range(NCORES)], axis=0)
    if "l3" not in _cache:
        _cache["l3"] = _build_launch3()
    r3 = _run(_cache["l3"],
              [{"h2": r2[c]["h2"], "st2": st2g, "g2": g2d, "be2": be2d,
                "w3t": W3T,
                "fmx": r1[c]["fm"] + np.float32(fm_bias + b3)}
               for c in range(NCORES)])

    out = np.concatenate(
        [np.ascontiguousarray(r3[c]["out"].T).reshape(BS)
         for c in range(NCORES)])
    return out.astype(np.float32)
